# revision 5
# baseline (speedup 1.0000x reference)
"""Trainium2 Bass kernel: dual-softmax cross-attention bilinear forms.

Math (per batch b, a = corr[b] in [N, N], N = 3072):
    attn = exp(2a) * (1/rowsum_a) outer (1/colsum_a)
    fund1 = v1^T attn v1,  fund2^T = v2^T attn^T v2
Device computes, per core (4 batches x 2 row-halves = 8 cores):
    E' = exp(a - 3) fp16 (scalar engine, rowsum via activation accum)
    E2 = E'^2 = exp(2a - 6) fp8e4 (vector engine)
    vr = v * e^6 / rowsumE fp8e4,  v = [x1 | x2 | pos] (pos shared)
    X  = E2^T @ vr  -- fp8 DoubleRow matmuls (2 contraction tiles/pass)
    colsumE partials via ones^T @ E' matmuls into 2 psum banks
Host finishes: colsum normalization + the small [N,262] bilinear GEMMs.

PSUM layout (8 banks): 5 banks = rotating [128,512] X accumulators
(x1-cols 0:256 start=True clears the bank; x2-cols 256:512 start=False
ride the per-element has_written bits), 1 bank = pos columns for all
24 m-tiles (pre-zeroed, start=False always, accumulated across both
chunks, DMA'd psum->dram at the end), 2 banks = colsum chunks.

Contraction is split in chunks of (3,3) ipairs only for the first M0
m-tiles so their GEMM overlaps the exp stream; the remaining m-tiles
run the full 6-ipair contraction after streaming ends. Chunk-0 X
partials park in SBUF fp16; chunk-1 exports add them back (vector),
full-contraction exports are plain copies (scalar).
"""

import numpy as np

import concourse.tile as tile
from concourse import bacc, bass_utils, mybir

B, N, C = 4, 3072, 256
H, W = 48, 64
CP = C + 6          # 262
CX = 2 * C          # 512: [x1 256 | x2 256]; pos 6 handled separately
NH = N // 2         # 1536 rows per core
NT = NH // 128      # 12 row tiles per core
NP = NT // 2        # 6 DoubleRow ipairs
MT = N // 128       # 24 column tiles
CS_CHUNK = 512
NCS = N // CS_CHUNK  # 6 colsum psum chunks

M0 = 12             # m-tiles whose contraction is split (2,4) chunks
PAIR_SPLIT = 3      # chunk-0 ipairs (of NP=6) for the split m-tiles

FP32 = mybir.dt.float32
FP16 = mybir.dt.float16
FP8 = mybir.dt.float8e4
DR = mybir.MatmulPerfMode.DoubleRow

EXP3 = float(np.exp(3.0))
EXP6 = float(np.exp(6.0))

TRACE = False
LAST_RESULT = None
_CACHED_NC = None


def _build_kernel():
    nc = bacc.Bacc("TRN2", target_bir_lowering=False, debug=False)
    a_in = nc.dram_tensor("a_half", [NH, N], FP16, kind="ExternalInput").ap()
    v_in = nc.dram_tensor("v_half", [NH, CX + 6], FP16, kind="ExternalInput").ap()
    x_out = nc.dram_tensor("x_out", [N, CX], FP16, kind="ExternalOutput").ap()
    pos_out = nc.dram_tensor("pos_out", [128, MT * 6], FP32, kind="ExternalOutput").ap()
    cs_out = nc.dram_tensor("cs_out", [8, CS_CHUNK], FP32, kind="ExternalOutput").ap()

    with tile.TileContext(nc) as tc:
        _kernel_body(tc, a_in, v_in, x_out, pos_out, cs_out)
    nc.compile()
    return nc


def _kernel_body(tc, a_in, v_in, x_out, pos_out, cs_out):
    nc = tc.nc
    with (
        tc.tile_pool(name="singles", bufs=1) as singles,
        tc.tile_pool(name="a_pool", bufs=4) as a_pool,
        tc.tile_pool(name="e_pool", bufs=4) as e_pool,
        tc.tile_pool(name="x_sb_pool", bufs=4) as x_sb_pool,
        tc.tile_pool(name="cs_psum", bufs=1, space="PSUM") as cs_psum,
        tc.tile_pool(name="pos_psum", bufs=1, space="PSUM") as pos_psum,
        tc.tile_pool(name="x_psum", bufs=5, space="PSUM") as x_psum,
    ):
        ones_t = singles.tile([128, 1], FP16)
        nc.vector.memset(ones_t, 1.0)
        bias_t = singles.tile([128, 1], FP32)
        nc.vector.memset(bias_t, -3.0)

        # prefetch the exp table-set off the critical path
        dummy_t = singles.tile([128, 1], FP32)
        nc.scalar.activation(
            out=dummy_t, in_=bias_t, func=mybir.ActivationFunctionType.Exp
        )

        v_sb = singles.tile([128, NT, CX + 6], FP16)
        vr_all = singles.tile([128, NT, CX + 6], FP8)
        e2_all = singles.tile([128, NT, N], FP8)
        rowsum_all = singles.tile([128, NT], FP32)
        rinv_all = singles.tile([128, NT], FP32)
        x_acc = singles.tile([128, M0, CX], FP16)

        # 6 colsum chunks packed into 2 psum banks at partitions 0/32/64/96.
        # Pre-zeroed; every matmul accumulates (start=False).
        cs_bank = [
            cs_psum.tile([128, CS_CHUNK], FP32, name=f"csb{t}", tag=f"csb{t}")
            for t in range(2)
        ]
        for t in range(2):
            nc.vector.memset(cs_bank[t], 0.0)

        def cs_ap(j):
            t, p = divmod(j, 4)
            return cs_bank[t][32 * p : 32 * p + 1, :]

        # pos columns for every m-tile live in one always-resident psum
        # bank, accumulated over all ipairs of both chunks (start=False).
        pos_pt = pos_psum.tile([128, MT, 6], FP32, name="pos_pt", tag="pos_pt")
        nc.vector.memset(pos_pt, 0.0)

        def stream_tile(i):
            a_t = a_pool.tile([128, N], FP16, name="a_t", tag="a_t")
            if i == 0:
                # split the first load across 4 DMA queues to cut the
                # cold-start latency before the first exp
                for q in range(4):
                    nc.sync.dma_start(
                        out=a_t[:, q * 768 : (q + 1) * 768],
                        in_=a_in[0:128, q * 768 : (q + 1) * 768],
                    )
            else:
                nc.sync.dma_start(
                    out=a_t, in_=a_in[i * 128 : (i + 1) * 128, :]
                )

            # E' = exp(a - 3) fp16; rowsum' accumulated per partition
            e_t = e_pool.tile([128, N], FP16, name="e_t", tag="e_t")
            nc.scalar.activation(
                out=e_t,
                in_=a_t,
                func=mybir.ActivationFunctionType.Exp,
                bias=bias_t,
                scale=1.0,
                accum_out=rowsum_all[:, i : i + 1],
            )

            # colsum partials: ones^T @ E', accumulated over all tiles
            for j in range(NCS):
                nc.tensor.matmul(
                    cs_ap(j),
                    lhsT=ones_t,
                    rhs=e_t[:, j * CS_CHUNK : (j + 1) * CS_CHUNK],
                    start=False,
                    stop=(i == NT - 1),
                    skip_group_check=True,
                    tile_position=(0, 32 * (j % 4)),
                )

            # E2 = E'^2 = exp(2a - 6), fp8, persistent until consumed
            nc.vector.tensor_mul(e2_all[:, i, :], e_t, e_t)

            # vr = (e^6 / rowsumE) * [x1|x2|pos]  (fp8)
            nc.sync.dma_start(
                out=v_sb[:, i, :], in_=v_in[i * 128 : (i + 1) * 128, :]
            )
            nc.vector.reciprocal(
                rinv_all[:, i : i + 1], rowsum_all[:, i : i + 1]
            )
            nc.vector.tensor_scalar(
                out=vr_all[:, i, :],
                in0=v_sb[:, i, :],
                scalar1=rinv_all[:, i : i + 1],
                scalar2=EXP6,
                op0=mybir.AluOpType.mult,
                op1=mybir.AluOpType.mult,
            )

        def gemm(m, p_lo, p_hi, xp):
            """Accumulate ipairs [p_lo, p_hi) of m's X into psum tile xp."""
            for p in range(p_lo, p_hi):
                lhsT = e2_all[:, 2 * p : 2 * p + 2, m * 128 : (m + 1) * 128]
                first = p == p_lo
                last = p == p_hi - 1
                nc.tensor.matmul(
                    xp[:, 0:256],
                    lhsT=lhsT,
                    rhs=vr_all[:, 2 * p : 2 * p + 2, 0:256],
                    start=first,
                    stop=last,
                    perf_mode=DR,
                    skip_group_check=True,
                )
                nc.tensor.matmul(
                    xp[:, 256:512],
                    lhsT=lhsT,
                    rhs=vr_all[:, 2 * p : 2 * p + 2, 256:512],
                    start=False,  # bank cleared by the 0:256 start
                    stop=last,
                    perf_mode=DR,
                    skip_group_check=True,
                )
                nc.tensor.matmul(
                    pos_pt[:, m, :],
                    lhsT=lhsT,
                    rhs=vr_all[:, 2 * p : 2 * p + 2, CX : CX + 6],
                    start=False,  # pre-zeroed resident bank
                    stop=(p == NP - 1),
                    perf_mode=DR,
                    skip_group_check=True,
                )

        def gemm_c0(m, engine):
            xp = x_psum.tile([128, CX], FP32, name="xp", tag="xp")
            gemm(m, 0, PAIR_SPLIT, xp)
            if engine == "v":
                nc.vector.tensor_copy(out=x_acc[:, m, :], in_=xp)
            else:
                nc.scalar.copy(out=x_acc[:, m, :], in_=xp)

        def gemm_tail(m):
            """Chunk-1 (for m < M0) or full contraction (m >= M0) + export."""
            xp = x_psum.tile([128, CX], FP32, name="xp", tag="xp")
            x_sb = x_sb_pool.tile([128, CX], FP16, name="x_sb", tag="x_sb")
            if m < M0:
                gemm(m, PAIR_SPLIT, NP, xp)
                nc.vector.tensor_add(x_sb, xp, x_acc[:, m, :])
            else:
                gemm(m, 0, NP, xp)
                nc.scalar.copy(out=x_sb, in_=xp)
            nc.sync.dma_start(
                out=x_out[m * 128 : (m + 1) * 128, :], in_=x_sb
            )

        # ---- phase A: stream chunk-0 tiles ----
        for i in range(2 * PAIR_SPLIT):
            stream_tile(i)

        # ---- phase B: stream chunk-1, interleave chunk-0 GEMM ----
        n_tail = NT - 2 * PAIR_SPLIT
        done = 0
        for k, i in enumerate(range(2 * PAIR_SPLIT, NT)):
            stream_tile(i)
            want = (k + 1) * M0 // n_tail
            for q, m in enumerate(range(done, want)):
                gemm_c0(m, "s" if q % 3 == 2 else "v")
            done = want

        # ---- colsum psum -> sbuf -> DRAM (only 4 used rows per bank) ----
        cs_sb = singles.tile([128, 2, CS_CHUNK], FP32)
        for t in range(2):
            nc.vector.tensor_copy(out=cs_sb[:, t, :], in_=cs_bank[t])
            nc.sync.dma_start(
                out=cs_out[4 * t : 4 * t + 4, :], in_=cs_sb[0:128:32, t, :]
            )

        # ---- phase D: chunk-1 + full-contraction GEMMs ----
        # interleave split-m (cheap) and full-m (expensive) for even flow
        order = []
        lo, hi = 0, M0
        for m in range(MT):
            if m % 2 == 0 and hi < MT:
                order.append(hi)
                hi += 1
            elif lo < M0:
                order.append(lo)
                lo += 1
            else:
                order.append(hi)
                hi += 1
        for m in order:
            gemm_tail(m)

        # pos psum -> sbuf -> DRAM
        pos_sb = singles.tile([128, MT * 6], FP32)
        nc.vector.tensor_copy(out=pos_sb, in_=pos_pt)
        nc.sync.dma_start(out=pos_out, in_=pos_sb)


def _positional_encodings():
    ys = np.linspace(-1.0, 1.0, H, dtype=np.float32)
    xs = np.linspace(-1.0, 1.0, W, dtype=np.float32)
    p3 = np.tile(ys, W)
    p4 = np.repeat(xs, H)
    pos = np.stack([p3 * p3, p4 * p4, p3 * p4, p3, p4, np.ones_like(p3)], axis=-1)
    return pos.astype(np.float32)  # [N, 6]


def kernel(x1, x2, corr, W_proj, b_proj):
    global _CACHED_NC, LAST_RESULT
    x1 = np.asarray(x1, dtype=np.float32)
    x2 = np.asarray(x2, dtype=np.float32)
    corr = np.asarray(corr, dtype=np.float32)
    W_proj = np.asarray(W_proj, dtype=np.float32)
    b_proj = np.asarray(b_proj, dtype=np.float32)

    pos = _positional_encodings()
    a = corr.reshape(B, N, N).astype(np.float16)
    # v = [x1 | x2 | pos] fp16 (pos block shared between the two GEMMs)
    v_all = np.concatenate(
        [x1, x2, np.broadcast_to(pos, (B, N, 6))], axis=2
    ).astype(np.float16)

    if _CACHED_NC is None:
        _CACHED_NC = _build_kernel()
    nc = _CACHED_NC

    in_maps = []
    for b in range(B):
        for h in range(2):
            rows = slice(h * NH, (h + 1) * NH)
            in_maps.append(
                {
                    "a_half": np.ascontiguousarray(a[b, rows, :]),
                    "v_half": np.ascontiguousarray(v_all[b, rows, :]),
                }
            )

    res = bass_utils.run_bass_kernel_spmd(
        nc, in_maps, core_ids=list(range(8)), trace=TRACE
    )
    LAST_RESULT = res

    v1 = np.concatenate([x1, np.broadcast_to(pos, (B, N, 6))], axis=2)
    v2 = np.concatenate([x2, np.broadcast_to(pos, (B, N, 6))], axis=2)

    out1 = np.empty((B, CP, C), dtype=np.float32)
    out2 = np.empty((B, CP, C), dtype=np.float32)
    inv_e6 = np.float32(1.0 / EXP6)
    for b in range(B):
        r0, r1 = res.results[2 * b], res.results[2 * b + 1]
        X = r0["x_out"].astype(np.float32) + r1["x_out"].astype(np.float32)
        # pos_out[p, m*6:(m+1)*6] holds X rows m*128+p
        pos_x = (
            r0["pos_out"].reshape(128, MT, 6) + r1["pos_out"].reshape(128, MT, 6)
        ).transpose(1, 0, 2).reshape(N, 6)
        # colsum chunks: rows 0-3 = bank0 chunks 0-3, rows 4-5 = chunks 4-5
        colsum = np.empty(N, dtype=np.float32)
        for j in range(NCS):
            t, p = divmod(j, 4)
            colsum[j * CS_CHUNK : (j + 1) * CS_CHUNK] = (
                r0["cs_out"][4 * t + p] + r1["cs_out"][4 * t + p]
            )
        c = inv_e6 / colsum
        vc1 = v1[b] * c[:, None]
        vc2 = v2[b] * c[:, None]
        X1 = np.concatenate([X[:, 0:256], pos_x], axis=1)   # [N, 262]
        X2 = np.concatenate([X[:, 256:512], pos_x], axis=1)
        fund1 = X1.T @ vc1      # [262, 262] = v1^T attn v1
        fund2t = X2.T @ vc2     # = (v2^T attn^T v2)^T
        out1[b] = fund1.T @ W_proj + b_proj
        out2[b] = fund2t @ W_proj + b_proj
    return (out2, out1)


# revision 16
# speedup vs baseline: 1.0265x; 1.0265x over previous
"""Trainium2 Bass kernel: dual-softmax cross-attention bilinear forms.

Math (per batch b, a = corr[b] in [N, N], N = 3072):
    attn = exp(2a) * (1/rowsum_a) outer (1/colsum_a)
    fund1 = v1^T attn v1,  fund2^T = v2^T attn^T v2
Device computes, per core (4 batches x 2 row-halves = 8 cores):
    E' = exp(a - 3) fp16 (scalar engine, rowsum via activation accum)
    E2 = E'^2 = exp(2a - 6) fp8e4 (vector engine)
    vr = v * e^6 / rowsumE fp8e4,  v = [x1 | x2 | pos]
    X  = E2^T @ vr  -- fp8 DoubleRow matmuls (256-row contraction/pass)
    Xpos^T = vr_pos^T @ E2 -- transposed so the 6-wide stationary makes
    the runt matmuls weight-load-free (moving = 512-elem e2 chunks)
    colsumE partials via ones^T @ E' matmuls into 2 psum banks
Host finishes: colsum normalization + the small [N,262] bilinear GEMMs.

PSUM (8 banks): 4 rotating [128,512] X accumulators (x1-cols 0:256
start=True clears the bank, x2-cols 256:512 start=False ride the
per-element has_written bits), 2 colsum banks, 2 pos^T banks (12
chunks of [6,256] at partition offsets 0/32/64, pre-zeroed,
start=False, accumulated across all ipairs).

Contraction is chunked (3,3 ipairs) only for the first M0 m-tiles so
their GEMM overlaps the exp stream; the rest run the full 6-ipair
contraction after streaming. Chunk-0 partials park in SBUF fp16
(vector copies); chunk-1 exports add them back (vector); full-
contraction exports are scalar copies.
"""

import numpy as np

import concourse.tile as tile
from concourse import bacc, bass_utils, mybir

B, N, C = 4, 3072, 256
H, W = 48, 64
CP = C + 6          # 262
CX = 2 * C          # 512: [x1 256 | x2 256]; pos 6 separate
NH = N // 2         # 1536 rows per core
NT = NH // 128      # 12 row tiles per core
NP = NT // 2        # 6 DoubleRow ipairs
MT = N // 128       # 24 column tiles
CS_CHUNK = 512
NCS = N // CS_CHUNK  # 6 colsum psum chunks
NPC = N // 256       # 12 pos^T column chunks

M0 = 16             # m-tiles whose contraction is split (3,3) ipairs
PAIR_SPLIT = 3      # chunk-0 ipairs (of NP=6) for the split m-tiles

FP32 = mybir.dt.float32
FP16 = mybir.dt.float16
FP8 = mybir.dt.float8e4
DR = mybir.MatmulPerfMode.DoubleRow
MUL = mybir.AluOpType.mult

EXP6 = float(np.exp(6.0))

TRACE = False
LAST_RESULT = None
_CACHED_NC = None


def _build_kernel():
    nc = bacc.Bacc("TRN2", target_bir_lowering=False, debug=False)
    a_in = nc.dram_tensor("a_half", [NH, N], FP16, kind="ExternalInput").ap()
    v_in = nc.dram_tensor("v_half", [NH, CX + 6], FP16, kind="ExternalInput").ap()
    x_out = nc.dram_tensor("x_out", [N, CX], FP16, kind="ExternalOutput").ap()
    pos_out = nc.dram_tensor("pos_out", [16, 3, 2 * CS_CHUNK], FP32, kind="ExternalOutput").ap()
    cs_out = nc.dram_tensor("cs_out", [8, CS_CHUNK], FP32, kind="ExternalOutput").ap()

    with tile.TileContext(nc) as tc:
        _kernel_body(tc, a_in, v_in, x_out, pos_out, cs_out)
    nc.compile()
    return nc


def _pos_slot(ch):
    """pos^T chunk ch (of 12) -> (round, bank t, col offset).

    DoubleRow matmul dst must sit at partition 0, so only 4 chunk slots
    (2 banks x 2 col halves) exist; the 12 chunks go through in 3
    rounds (round 0 during the stream, rounds 1-2 replayed in phase D).
    """
    r, s = divmod(ch, 4)
    return r, s // 2, 256 * (s % 2)


def _kernel_body(tc, a_in, v_in, x_out, pos_out, cs_out):
    nc = tc.nc
    with (
        tc.tile_pool(name="singles", bufs=1) as singles,
        tc.tile_pool(name="a_pool", bufs=4) as a_pool,
        tc.tile_pool(name="e_pool", bufs=6) as e_pool,
        tc.tile_pool(name="x_sb_pool", bufs=4) as x_sb_pool,
        tc.tile_pool(name="cs_psum", bufs=1, space="PSUM") as cs_psum,
        tc.tile_pool(name="pos_psum", bufs=1, space="PSUM") as pos_psum,
        tc.tile_pool(name="x_psum", bufs=4, space="PSUM") as x_psum,
    ):
        ones_t = singles.tile([128, 1], FP16)
        nc.vector.memset(ones_t, 1.0)
        bias_t = singles.tile([128, 1], FP32)
        nc.vector.memset(bias_t, -3.0)

        # prefetch the exp table-set off the critical path
        dummy_t = singles.tile([128, 1], FP32)
        nc.scalar.activation(
            out=dummy_t, in_=bias_t, func=mybir.ActivationFunctionType.Exp
        )

        v_sb = singles.tile([128, NT, CX + 6], FP16)
        vr_all = singles.tile([128, NT, CX], FP8)
        # pos columns padded to stride 16 (fp8 ldweights: step % 16 == 0)
        vr_pos = singles.tile([128, NT, 16], FP8)
        nc.vector.memset(vr_pos, 0.0)
        e2_all = singles.tile([128, NT, N], FP8)
        rowsum_all = singles.tile([128, NT], FP32)
        rinv_all = singles.tile([128, NT], FP32)
        x_acc = singles.tile([128, M0, CX], FP16)
        pos_sb = singles.tile([128, 3, 2 * CS_CHUNK], FP32)

        # 6 colsum chunks packed into 2 psum banks at partitions 0/32/64/96.
        # Pre-zeroed; every matmul accumulates (start=False).
        cs_bank = [
            cs_psum.tile([128, CS_CHUNK], FP32, name=f"csb{t}", tag=f"csb{t}")
            for t in range(2)
        ]
        for t in range(2):
            nc.vector.memset(cs_bank[t], 0.0)

        def cs_ap(j):
            t, p = divmod(j, 4)
            return cs_bank[t][32 * p : 32 * p + 1, :]

        # pos^T chunks: 12 x [6, 256] over 2 pre-zeroed banks
        pos_bank = [
            pos_psum.tile([128, CS_CHUNK], FP32, name=f"posb{t}", tag=f"posb{t}")
            for t in range(2)
        ]
        for t in range(2):
            nc.vector.memset(pos_bank[t], 0.0)

        def stream_tile(i):
            a_t = a_pool.tile([128, N], FP16, name="a_t", tag="a_t")
            if i == 0:
                # split the first load across 4 DMA queues to cut the
                # cold-start latency before the first exp
                for q in range(4):
                    nc.sync.dma_start(
                        out=a_t[:, q * 768 : (q + 1) * 768],
                        in_=a_in[0:128, q * 768 : (q + 1) * 768],
                    )
            else:
                nc.sync.dma_start(
                    out=a_t, in_=a_in[i * 128 : (i + 1) * 128, :]
                )

            # E' = exp(a - 3) fp16; rowsum' accumulated per partition
            e_t = e_pool.tile([128, N], FP16, name="e_t", tag="e_t")
            nc.scalar.activation(
                out=e_t,
                in_=a_t,
                func=mybir.ActivationFunctionType.Exp,
                bias=bias_t,
                scale=1.0,
                accum_out=rowsum_all[:, i : i + 1],
            )

            # colsum partials: ones^T @ E', accumulated over all tiles
            for j in range(NCS):
                nc.tensor.matmul(
                    cs_ap(j),
                    lhsT=ones_t,
                    rhs=e_t[:, j * CS_CHUNK : (j + 1) * CS_CHUNK],
                    start=False,
                    stop=(i == NT - 1),
                    skip_group_check=True,
                    tile_position=(0, 32 * (j % 4)),
                )

            # E2 = E'^2 = exp(2a - 6), fp8, persistent until consumed
            nc.vector.scalar_tensor_tensor(
                out=e2_all[:, i, :],
                in0=e_t,
                scalar=1.0,
                in1=e_t,
                op0=MUL,
                op1=MUL,
            )

            # vr = (e^6 / rowsumE) * [x1|x2|pos]  (fp8)
            nc.sync.dma_start(
                out=v_sb[:, i, :], in_=v_in[i * 128 : (i + 1) * 128, :]
            )
            nc.vector.reciprocal(
                rinv_all[:, i : i + 1], rowsum_all[:, i : i + 1]
            )
            nc.vector.tensor_scalar(
                out=vr_all[:, i, :],
                in0=v_sb[:, i, 0:CX],
                scalar1=rinv_all[:, i : i + 1],
                scalar2=EXP6,
                op0=MUL,
                op1=MUL,
            )
            nc.vector.tensor_scalar(
                out=vr_pos[:, i, 0:6],
                in0=v_sb[:, i, CX : CX + 6],
                scalar1=rinv_all[:, i : i + 1],
                scalar2=EXP6,
                op0=MUL,
                op1=MUL,
            )

        def pos_gemm(p, rnd):
            """Xpos^T += vr_pos_pair^T @ e2_pair for round rnd's 4 chunks."""
            lhsT = vr_pos[:, 2 * p : 2 * p + 2, :]
            for ch in range(4 * rnd, 4 * rnd + 4):
                _, t, pcol = _pos_slot(ch)
                nc.tensor.matmul(
                    pos_bank[t][0:16, pcol : pcol + 256],
                    lhsT=lhsT,
                    rhs=e2_all[:, 2 * p : 2 * p + 2, ch * 256 : (ch + 1) * 256],
                    start=False,
                    stop=(p == NP - 1),
                    perf_mode=DR,
                    skip_group_check=True,
                )

        def pos_export(rnd):
            for t in range(2):
                nc.vector.tensor_copy(
                    out=pos_sb[:, rnd, 512 * t : 512 * (t + 1)],
                    in_=pos_bank[t],
                )
                if rnd < 2:
                    nc.vector.memset(pos_bank[t], 0.0)

        def gemm(m, p_lo, p_hi, xp):
            """Accumulate ipairs [p_lo, p_hi) of m's X into psum tile xp."""
            for p in range(p_lo, p_hi):
                lhsT = e2_all[:, 2 * p : 2 * p + 2, m * 128 : (m + 1) * 128]
                first = p == p_lo
                last = p == p_hi - 1
                nc.tensor.matmul(
                    xp[:, 0:256],
                    lhsT=lhsT,
                    rhs=vr_all[:, 2 * p : 2 * p + 2, 0:256],
                    start=first,
                    stop=last,
                    perf_mode=DR,
                    skip_group_check=True,
                )
                nc.tensor.matmul(
                    xp[:, 256:512],
                    lhsT=lhsT,
                    rhs=vr_all[:, 2 * p : 2 * p + 2, 256:512],
                    start=False,  # bank cleared by the 0:256 start
                    stop=last,
                    perf_mode=DR,
                    skip_group_check=True,
                )

        def gemm_c0(m):
            xp = x_psum.tile([128, CX], FP32, name="xp", tag="xp")
            gemm(m, 0, PAIR_SPLIT, xp)
            nc.vector.tensor_copy(out=x_acc[:, m, :], in_=xp)

        def gemm_tail(m):
            """Chunk-1 (for m < M0) or full contraction (m >= M0) + export."""
            xp = x_psum.tile([128, CX], FP32, name="xp", tag="xp")
            x_sb = x_sb_pool.tile([128, CX], FP16, name="x_sb", tag="x_sb")
            if m < M0:
                gemm(m, PAIR_SPLIT, NP, xp)
                nc.vector.tensor_add(x_sb, xp, x_acc[:, m, :])
            else:
                gemm(m, 0, NP, xp)
                nc.scalar.copy(out=x_sb, in_=xp)
            nc.sync.dma_start(
                out=x_out[m * 128 : (m + 1) * 128, :], in_=x_sb
            )

        # ---- phase A: stream chunk-0 tiles; pos^T round 0 fills idle PE ----
        for i in range(2 * PAIR_SPLIT):
            stream_tile(i)
            if i % 2 == 1:
                pos_gemm(i // 2, 0)

        # ---- phase B: stream chunk-1, interleave chunk-0 GEMM ----
        n_tail = NT - 2 * PAIR_SPLIT
        done = 0
        for k, i in enumerate(range(2 * PAIR_SPLIT, NT)):
            stream_tile(i)
            if i % 2 == 1:
                pos_gemm(i // 2, 0)
            want = (k + 1) * M0 // n_tail
            for m in range(done, want):
                gemm_c0(m)
            done = want

        # ---- colsum psum -> sbuf -> DRAM (4 used rows per bank) ----
        cs_sb = singles.tile([128, 2, CS_CHUNK], FP32)
        for t in range(2):
            nc.vector.tensor_copy(out=cs_sb[:, t, :], in_=cs_bank[t])
            nc.sync.dma_start(
                out=cs_out[4 * t : 4 * t + 4, :], in_=cs_sb[0:128:32, t, :]
            )

        # ---- phase D: chunk-1 + full-contraction GEMMs ----
        # interleave split-m (cheap) and full-m (expensive) for even flow
        order = []
        lo, hi = 0, M0
        for m in range(MT):
            if m % 3 == 1 and hi < MT:
                order.append(hi)
                hi += 1
            elif lo < M0:
                order.append(lo)
                lo += 1
            else:
                order.append(hi)
                hi += 1
        for idx, m in enumerate(order):
            gemm_tail(m)
            # replay pos^T rounds 1-2 once the psum slots have drained
            if idx == 5:
                pos_export(0)
            if idx == 7:
                for p in range(NP):
                    pos_gemm(p, 1)
            if idx == 15:
                pos_export(1)
            if idx == 17:
                for p in range(NP):
                    pos_gemm(p, 2)
        pos_export(2)
        nc.sync.dma_start(out=pos_out, in_=pos_sb[0:16, :, :])


def _positional_encodings():
    ys = np.linspace(-1.0, 1.0, H, dtype=np.float32)
    xs = np.linspace(-1.0, 1.0, W, dtype=np.float32)
    p3 = np.tile(ys, W)
    p4 = np.repeat(xs, H)
    pos = np.stack([p3 * p3, p4 * p4, p3 * p4, p3, p4, np.ones_like(p3)], axis=-1)
    return pos.astype(np.float32)  # [N, 6]


def kernel(x1, x2, corr, W_proj, b_proj):
    global _CACHED_NC, LAST_RESULT
    x1 = np.asarray(x1, dtype=np.float32)
    x2 = np.asarray(x2, dtype=np.float32)
    corr = np.asarray(corr, dtype=np.float32)
    W_proj = np.asarray(W_proj, dtype=np.float32)
    b_proj = np.asarray(b_proj, dtype=np.float32)

    pos = _positional_encodings()
    a = corr.reshape(B, N, N).astype(np.float16)
    # v = [x1 | x2 | pos] fp16 (pos block shared between the two GEMMs)
    v_all = np.concatenate(
        [x1, x2, np.broadcast_to(pos, (B, N, 6))], axis=2
    ).astype(np.float16)

    if _CACHED_NC is None:
        _CACHED_NC = _build_kernel()
    nc = _CACHED_NC

    in_maps = []
    for b in range(B):
        for h in range(2):
            rows = slice(h * NH, (h + 1) * NH)
            in_maps.append(
                {
                    "a_half": np.ascontiguousarray(a[b, rows, :]),
                    "v_half": np.ascontiguousarray(v_all[b, rows, :]),
                }
            )

    res = bass_utils.run_bass_kernel_spmd(
        nc, in_maps, core_ids=list(range(8)), trace=TRACE
    )
    LAST_RESULT = res

    v1 = np.concatenate([x1, np.broadcast_to(pos, (B, N, 6))], axis=2)
    v2 = np.concatenate([x2, np.broadcast_to(pos, (B, N, 6))], axis=2)

    out1 = np.empty((B, CP, C), dtype=np.float32)
    out2 = np.empty((B, CP, C), dtype=np.float32)
    inv_e6 = np.float32(1.0 / EXP6)
    for b in range(B):
        r0, r1 = res.results[2 * b], res.results[2 * b + 1]
        X = r0["x_out"].astype(np.float32) + r1["x_out"].astype(np.float32)
        # decode pos^T chunks: posT[0:6, ch*256:(ch+1)*256] from round slots
        pos_raw = r0["pos_out"] + r1["pos_out"]   # [16, 3, 1024]
        posT = np.empty((6, N), dtype=np.float32)
        for ch in range(NPC):
            r, t, pcol = _pos_slot(ch)
            posT[:, ch * 256 : (ch + 1) * 256] = pos_raw[
                0:6, r, 512 * t + pcol : 512 * t + pcol + 256
            ]
        pos_x = posT.T                             # [N, 6]
        # colsum chunks: rows 0-3 = bank0 chunks 0-3, rows 4-5 = chunks 4-5
        colsum = np.empty(N, dtype=np.float32)
        for j in range(NCS):
            t, p = divmod(j, 4)
            colsum[j * CS_CHUNK : (j + 1) * CS_CHUNK] = (
                r0["cs_out"][4 * t + p] + r1["cs_out"][4 * t + p]
            )
        c = inv_e6 / colsum
        vc1 = v1[b] * c[:, None]
        vc2 = v2[b] * c[:, None]
        X1 = np.concatenate([X[:, 0:256], pos_x], axis=1)   # [N, 262]
        X2 = np.concatenate([X[:, 256:512], pos_x], axis=1)
        fund1 = X1.T @ vc1      # [262, 262] = v1^T attn v1
        fund2t = X2.T @ vc2     # = (v2^T attn^T v2)^T
        out1[b] = fund1.T @ W_proj + b_proj
        out2[b] = fund2t @ W_proj + b_proj
    return (out2, out1)
